# revision 1
# baseline (speedup 1.0000x reference)
"""AttenNetVLAD Trainium2 kernel (8-core data parallel).

Reference computation per batch n (C=512 channels, P=1600 pixels, K=64 clusters):
  hmp   = relu(attn_w . relu(x) + attn_b)                    # [P]
  xn    = x / max(||x||_c, eps)                              # [C,P]
  sa    = softmax_k(conv_w @ xn)                             # [K,P]
  w     = sa * hmp
  vlad  = l2norm_glob(l2norm_c(w @ xn^T - (w.1) * centroids))

Strategy: batch data-parallel over 8 cores (6 batches each). Per batch:
  - SWDGE DMA loads x fp32->bf16 into [c,p] tiles (cast rides the DMA).
  - One fused xbar DMA-transpose (bf16) produces xT [p,c] tiles (mapping:
    src col j -> partition j%128, chunk j//128 -- verified on HW).
  - norm2 via ACT Square+accum_out (NA chunks) and DVE STT (rest); hmp via
    DVE scalar_tensor_tensor (relu * attn_w)+accum_out, all from xT.
  - logitsT [p,k] on PE into one 2-bank PSUM tile: x [c,p] slices
    stationary, conv_w^T moving (bf16, fp32 PSUM accumulate).
  - softmax as 4 big ops: DVE mult by inv_norm (free-broadcast AP), one ACT
    Exp, one DVE multi-dim reduce for sumexp, one DVE broadcast-mult for
    w~ = exp * (hmp*inv_norm/sumexp); the extra inv_norm folds xn's
    normalization into w~ so term1 can use raw x.
  - term1 [k,c] + wsum on PE contracting p (w~ stationary, xT moving); the
    wsum matmul's rhs is the bf16 norm column, cancelling w~'s inv_norm.
  - all rsqrts via magic-seed Newton on the idle GPSIMD engine, keeping
    ACT's table set at {Square, Exp, Copy} (zero activation-table reloads).
  - vlad assembly + both l2 norms as per-partition scales; sign of
    (wsum*cen - term1) fixed by negating the final scale.
Loads/transposes are issued in batch pairs to halve DMA phase switches
(xbar transposes serialize against other DMA traffic).
"""

import numpy as np
from contextlib import ExitStack

import concourse.bass as bass
import concourse.bacc as bacc
import concourse.bass_isa as bass_isa
import concourse.tile as tile
from concourse import mybir
from concourse.bass_utils import run_bass_kernel_spmd

F32 = mybir.dt.float32
BF16 = mybir.dt.bfloat16
I32 = mybir.dt.int32
ALU = mybir.AluOpType
ACTF = mybir.ActivationFunctionType

N_CORES = 8
NB = 6            # batches per core
C = 512
P = 1600
K = 64
CB = 4            # channel blocks of 128
PCH = 13          # p chunks of 128
PPAD = PCH * 128  # 1664
NA = 12           # norm2 chunks on ACT (rest on DVE) -- cost-model tuned
PAIR = 1          # batches per load/transpose group (2 modeled worse)

_CACHE = {}

_RSQRT_MAGIC = 0x5F3759DF


def _rsqrt_newton(nc, pool, src_ap, shape, tag):
    """1/sqrt(src): magic-constant seed (DVE, needs shift) + 2 Newton
    iterations on GPSIMD. src must be positive (clamp upstream). Returns an
    fp32 AP (bitcast view of an int32 tile)."""
    g = nc.gpsimd
    yi = pool.tile(shape, I32, tag=tag + "_yi")
    t0 = pool.tile(shape, F32, tag=tag + "_t0")
    si = src_ap.bitcast(I32)
    nc.vector.tensor_scalar(out=yi, in0=si, scalar1=1, scalar2=None,
                            op0=ALU.arith_shift_right)
    nc.vector.tensor_scalar(out=yi, in0=yi, scalar1=-1, scalar2=_RSQRT_MAGIC,
                            op0=ALU.mult, op1=ALU.add)
    yv = yi[tuple([slice(None)] * len(shape))].bitcast(F32)
    for _ in range(2):
        g.tensor_tensor(out=t0, in0=yv, in1=yv, op=ALU.mult)
        g.tensor_tensor(out=t0, in0=t0, in1=src_ap, op=ALU.mult)
        g.tensor_scalar(out=t0, in0=t0, scalar1=-0.5, scalar2=1.5,
                        op0=ALU.mult, op1=ALU.add)
        g.tensor_tensor(out=yv, in0=yv, in1=t0, op=ALU.mult)
    return yv


def _bcast_ap(handle_ap, parts, free_ap):
    return bass.AP(tensor=handle_ap.tensor, offset=handle_ap.offset,
                   ap=[[0, parts]] + free_ap)


def _build():
    nc = bacc.Bacc("TRN2", target_bir_lowering=False, debug=False,
                   num_devices=N_CORES)
    x_in = nc.declare_dram_parameter("x", [NB, C, P], F32, isOutput=False)
    cw_in = nc.declare_dram_parameter("conv_w", [K, C], F32, isOutput=False)
    aw_in = nc.declare_dram_parameter("attn_w", [1, C], F32, isOutput=False)
    ab_in = nc.declare_dram_parameter("attn_b", [1], F32, isOutput=False)
    cen_in = nc.declare_dram_parameter("centroids", [K, C], F32, isOutput=False)
    out_p = nc.declare_dram_parameter("out", [NB, K * C], F32, isOutput=True)
    out_v = out_p[:, :].rearrange("n (k c) -> n k c", k=K)

    with tile.TileContext(nc) as tc, ExitStack() as ctx:
        const = ctx.enter_context(tc.tile_pool(name="const", bufs=1))
        import os as _o
        big = ctx.enter_context(tc.tile_pool(name="big", bufs=2))
        bigt = ctx.enter_context(tc.tile_pool(name="bigt", bufs=2))
        med = ctx.enter_context(tc.tile_pool(name="med", bufs=3))
        gp = ctx.enter_context(tc.tile_pool(name="gp", bufs=3))
        ps_log = ctx.enter_context(tc.tile_pool(name="ps_log", bufs=2, space="PSUM"))
        ps_t1 = ctx.enter_context(tc.tile_pool(name="ps_t1", bufs=2, space="PSUM"))
        ps_ws = ctx.enter_context(tc.tile_pool(name="ps_ws", bufs=2, space="PSUM"))

        # ---- constants ----
        cw_f = const.tile([K, C], F32)
        nc.sync.dma_start(out=cw_f, in_=cw_in[:, :])
        cw_b = const.tile([K, C], BF16)
        nc.vector.tensor_copy(cw_b, cw_f)
        # cwT[cc, cb, k] = conv_w[k, cb*128+cc] via PE transpose (keeps the
        # DMA chain free of extra exclusive xbar windows at startup)
        eye = const.tile([K, K], BF16)
        nc.vector.memset(eye, 1.0)
        nc.gpsimd.affine_select(out=eye, in_=eye, pattern=[[-1, K]],
                                compare_op=ALU.is_equal, fill=0.0,
                                base=0, channel_multiplier=1)
        cwT = const.tile([128, CB, K], BF16)
        for cb in range(CB):
            pst = ps_t1.tile([128, K], BF16, tag="t1")
            nc.tensor.transpose(pst, cw_b[:, cb * 128:(cb + 1) * 128], eye)
            nc.scalar.activation(out=cwT[:, cb, :], in_=pst, func=ACTF.Copy)
        awB = const.tile([128, CB, 128], F32)  # attn_w broadcast to all partitions
        nc.gpsimd.dma_start(out=awB, in_=_bcast_ap(aw_in[:, :], 128, [[128, CB], [1, 128]]))
        bB = const.tile([128, 1], F32)
        nc.gpsimd.dma_start(out=bB, in_=_bcast_ap(ab_in[:], 128, [[1, 1]]))
        cen = const.tile([K, C], F32)
        nc.sync.dma_start(out=cen, in_=cen_in[:, :])

        def load_pair(bs, split=False):
            # two batches share one tile: ONE cast-load covers both (the
            # (h, cb) slot stride is uniformly 128*1600 across the pair) and
            # a single fused xbar transpose covers both.
            xb2 = big.tile([128, 2 * CB, PPAD], BF16, tag="xb")
            nc.vector.memset(xb2[:, :, P:PPAD], 0.0)
            if split:
                for h, b in enumerate(bs):
                    nc.gpsimd.dma_start(
                        out=xb2[:, h * CB:(h + 1) * CB, 0:P],
                        in_=x_in[b].rearrange("(cb cc) p -> cc cb p", cc=128))
            else:
                nc.gpsimd.dma_start(
                    out=xb2[:, :, 0:P],
                    in_=x_in[bs[0]:bs[0] + 2].rearrange(
                        "n (cb cc) p -> cc (n cb) p", cc=128))
            return xb2

        def transpose_pair(xb2, split=False):
            # xt2[pp, h, cb, ch, cc] = x_h[cb*128+cc, ch*128+pp]: src col j of
            # the flattened [128, 2*CB*PPAD] input lands at
            # out[j%128, j//128] = (pp, h*CB*PCH + cb*PCH + ch).
            xt2 = bigt.tile([128, 2, CB, PCH, 128], BF16, tag="xt")
            if split:
                half = CB * PPAD
                nc.sync.dma_start_transpose(out=xt2[:, 0], in_=xb2[:, 0:CB, :])
                nc.sync.dma_start_transpose(out=xt2[:, 1], in_=xb2[:, CB:2*CB, :])
            else:
                nc.sync.dma_start_transpose(out=xt2, in_=xb2[:, :, :])
            return xt2

        def compute_front(b, xb, xt, h=0):
            # ---- norm2 (split ACT/DVE) and hmp (DVE) from xT ----
            norm2 = med.tile([128, PCH], F32, tag="n2")
            hmp0 = med.tile([128, PCH], F32, tag="h0")
            junkA = med.tile([128, CB, 128], BF16, tag="jA")
            junkD = med.tile([128, CB, 128], BF16, tag="jD")
            import os as _oo
            _na = 11
            _ng = 0
            _hg = 0
            junkG = med.tile([128, CB, 128], BF16, tag="jG")
            _hmp_first = False
            _hsplit = False
            _hsplit2 = False
            def _emit_norm2():
                for ch in range(PCH):
                    if _hsplit:
                        if h == 0:
                            nc.scalar.activation(out=junkA, in_=xt[:, :, ch, :],
                                                 func=ACTF.Square,
                                                 accum_out=norm2[:, ch:ch + 1])
                        else:
                            nc.vector.scalar_tensor_tensor(
                                out=junkD, in0=xt[:, :, ch, :], scalar=1.0,
                                in1=xt[:, :, ch, :], op0=ALU.mult, op1=ALU.mult,
                                accum_out=norm2[:, ch:ch + 1])
                        continue
                    if ch < _na:
                        nc.scalar.activation(out=junkA, in_=xt[:, :, ch, :],
                                             func=ACTF.Square,
                                             accum_out=norm2[:, ch:ch + 1])
                    elif ch < _na + _ng:
                        nc.gpsimd.scalar_tensor_tensor(
                            out=junkG, in0=xt[:, :, ch, :], scalar=1.0,
                            in1=xt[:, :, ch, :], op0=ALU.mult, op1=ALU.mult,
                            accum_out=norm2[:, ch:ch + 1])
                    else:
                        nc.vector.scalar_tensor_tensor(
                            out=junkD, in0=xt[:, :, ch, :], scalar=1.0,
                            in1=xt[:, :, ch, :], op0=ALU.mult, op1=ALU.mult,
                            accum_out=norm2[:, ch:ch + 1])
            def _emit_hmp():
                for ch in range(PCH):
                    if _hsplit:
                        eng = nc.gpsimd if (h == 1 and _hsplit2) else nc.vector
                        jj = junkG if (h == 1 and _hsplit2) else junkD
                        eng.scalar_tensor_tensor(
                            out=jj, in0=xt[:, :, ch, :], scalar=0.0, in1=awB,
                            op0=ALU.max, op1=ALU.mult,
                            accum_out=hmp0[:, ch:ch + 1])
                        continue
                    if ch < _hg:
                        nc.gpsimd.scalar_tensor_tensor(
                            out=junkG, in0=xt[:, :, ch, :], scalar=0.0, in1=awB,
                            op0=ALU.max, op1=ALU.mult, accum_out=hmp0[:, ch:ch + 1])
                    else:
                        nc.vector.scalar_tensor_tensor(
                            out=junkD, in0=xt[:, :, ch, :], scalar=0.0, in1=awB,
                            op0=ALU.max, op1=ALU.mult, accum_out=hmp0[:, ch:ch + 1])
            if _hmp_first:
                _emit_hmp(); _emit_norm2()
            else:
                _emit_norm2(); _emit_hmp()

            nc.gpsimd.tensor_scalar_max(norm2, norm2, 1e-24)
            if False:
                _ln = med.tile([128, PCH], F32, tag="lninv")
                nc.scalar.activation(out=_ln, in_=norm2, func=ACTF.Ln)
                _iv = med.tile([128, PCH], F32, tag="ivn")
                nc.scalar.activation(out=_iv, in_=_ln, func=ACTF.Exp, scale=-0.5)
                invn = _iv[:, :]
            else:
                invn = _rsqrt_newton(nc, gp, norm2[:, :], [128, PCH], "invn")
            # bf16 norm column: wsum's matmul rhs, cancelling the inv_norm
            # folded into w~ (wsum = sum_p sa*hmp has no inv_norm).
            nrmb = med.tile([128, PCH], BF16, tag="nrmb")
            nc.gpsimd.tensor_tensor(out=nrmb, in0=norm2, in1=invn, op=ALU.mult)

            # ---- logitsT on PE into one 2-bank PSUM tile ----
            lps = ps_log.tile([128, PCH, K], F32, tag="lps")
            for ch in range(PCH):
                for cb in range(CB):
                    nc.tensor.matmul(lps[:, ch, :],
                                     xb[:, cb, ch * 128:(ch + 1) * 128],
                                     cwT[:, cb, :],
                                     start=(cb == 0), stop=(cb == CB - 1))

            # ---- softmax numerator/denominator as 4 big ops ----
            zs = big.tile([128, PCH, K], F32, tag="zs")
            nc.vector.tensor_tensor(out=zs, in0=lps,
                                    in1=invn.to_broadcast([128, PCH, K]),
                                    op=ALU.mult)
            expw = big.tile([128, PCH, K], F32, tag="expw")
            nc.scalar.activation(out=expw, in_=zs, func=ACTF.Exp)
            sume = med.tile([128, PCH], F32, tag="sume")
            nc.vector.tensor_reduce(out=sume, in_=expw,
                                    axis=mybir.AxisListType.X, op=ALU.add)

            # ---- srow = hmp * invn / sumexp ; w~ = expw * srow (bf16) ----
            hmp = gp.tile([128, PCH], F32, tag="hmp")
            nc.gpsimd.tensor_scalar(out=hmp, in0=hmp0, scalar1=bB, scalar2=0.0,
                                    op0=ALU.add, op1=ALU.max)
            hi = gp.tile([128, PCH], F32, tag="hi")
            nc.gpsimd.tensor_tensor(out=hi, in0=hmp, in1=invn, op=ALU.mult)
            srow = gp.tile([128, PCH], F32, tag="srow")
            if False:
                nc.vector.tensor_tensor(out=srow, in0=hi, in1=sume, op=ALU.divide)
            else:
                rcs = med.tile([128, PCH], F32, tag="rcs")
                nc.vector.reciprocal(rcs, sume)
                nc.gpsimd.tensor_tensor(out=srow, in0=hi, in1=rcs, op=ALU.mult)
            wt = med.tile([128, PCH, K], BF16, tag="wt")
            nc.vector.tensor_tensor(out=wt, in0=expw,
                                    in1=srow[:, :].to_broadcast([128, PCH, K]),
                                    op=ALU.mult)

            return wt, nrmb

        def compute_back(b, xt, wt, nrmb):
            import os as _oo
            # ---- term1 [k,c] and wsum [k] on PE (contract p) ----
            t1 = ps_t1.tile([K, C], F32, tag="t1")
            ws = ps_ws.tile([K, 1], F32, tag="ws")
            for ch in range(PCH):
                nc.tensor.matmul(t1, wt[:, ch, :], xt[:, :, ch, :],
                                 start=(ch == 0), stop=(ch == PCH - 1))
                nc.tensor.matmul(ws, wt[:, ch, :], nrmb[:, ch:ch + 1],
                                 start=(ch == 0), stop=(ch == PCH - 1))

            # ---- vlad assembly + normalization ----
            vneg = med.tile([K, C], F32, tag="vneg")   # wsum*cen - term1
            nc.vector.scalar_tensor_tensor(out=vneg, in0=cen, scalar=ws, in1=t1,
                                           op0=ALU.mult, op1=ALU.subtract)
            junkK = med.tile([K, C], BF16, tag="jK")
            ssq = med.tile([K, 1], F32, tag="ssq")
            nc.scalar.activation(out=junkK, in_=vneg, func=ACTF.Square,
                                 accum_out=ssq)
            nc.gpsimd.tensor_scalar_max(ssq, ssq, 1e-24)
            if False:
                _l1 = med.tile([K, 1], F32, tag="l1")
                nc.scalar.activation(out=_l1, in_=ssq, func=ACTF.Ln)
                _i1 = med.tile([K, 1], F32, tag="i1")
                nc.scalar.activation(out=_i1, in_=_l1, func=ACTF.Exp, scale=-0.5)
                inv1 = _i1[:, :]
            else:
                inv1 = _rsqrt_newton(nc, gp, ssq[:, :], [K, 1], "inv1")
            # global norm: sum_k of per-row normalized squares
            rn2 = gp.tile([K, 1], F32, tag="rn2")
            nc.gpsimd.tensor_tensor(out=rn2, in0=ssq, in1=inv1, op=ALU.mult)
            nc.gpsimd.tensor_tensor(out=rn2, in0=rn2, in1=inv1, op=ALU.mult)
            g2 = gp.tile([K, 1], F32, tag="g2")
            nc.gpsimd.partition_all_reduce(g2, rn2, channels=K,
                                           reduce_op=bass_isa.ReduceOp.add)
            if False:
                _lg = med.tile([K, 1], F32, tag="lg")
                nc.scalar.activation(out=_lg, in_=g2, func=ACTF.Ln)
                _ig = med.tile([K, 1], F32, tag="ig")
                nc.scalar.activation(out=_ig, in_=_lg, func=ACTF.Exp, scale=-0.5)
                invg = _ig[:, :]
            else:
                invg = _rsqrt_newton(nc, gp, g2[:, :], [K, 1], "invg")
            sfin = gp.tile([K, 1], F32, tag="sfin")   # -(inv1*invg) fixes sign
            nc.gpsimd.tensor_tensor(out=sfin, in0=inv1, in1=invg, op=ALU.mult)
            nc.gpsimd.tensor_scalar_mul(sfin, sfin, -1.0)
            outb = med.tile([K, C], F32, tag="outb")
            nc.scalar.activation(out=outb, in_=vneg, func=ACTF.Copy, scale=sfin)
            nc.sync.dma_start(out=out_v[b], in_=outb)

        for bp in range(0, NB, 2):
            bs = list(range(bp, bp + 2))
            xb2 = load_pair(bs)
            xt2 = transpose_pair(xb2, split=(bp == 0))
            xb2v = xb2.rearrange("q (h cb) p -> q h cb p", h=2)
            fronts = [compute_front(b, xb2v[:, h], xt2[:, h], h=h)
                      for h, b in enumerate(bs)]
            for h, b in enumerate(bs):
                wt, nrmb = fronts[h]
                compute_back(b, xt2[:, h], wt, nrmb)

    nc.finalize()
    return nc


def kernel(x, conv_w, attn_w, attn_b, centroids):
    x = np.ascontiguousarray(np.asarray(x, dtype=np.float32)).reshape(48, C, P)
    conv_w = np.ascontiguousarray(np.asarray(conv_w, dtype=np.float32))
    attn_w = np.ascontiguousarray(np.asarray(attn_w, dtype=np.float32)).reshape(1, C)
    attn_b = np.ascontiguousarray(np.asarray(attn_b, dtype=np.float32)).reshape(1)
    centroids = np.ascontiguousarray(np.asarray(centroids, dtype=np.float32))

    if "nc" not in _CACHE:
        _CACHE["nc"] = _build()
    nc = _CACHE["nc"]

    in_maps = []
    for i in range(N_CORES):
        in_maps.append({
            "x": x[i * NB:(i + 1) * NB],
            "conv_w": conv_w,
            "attn_w": attn_w,
            "attn_b": attn_b,
            "centroids": centroids,
        })
    res = run_bass_kernel_spmd(nc, in_maps, list(range(N_CORES)))
    out = np.concatenate([res.results[i]["out"] for i in range(N_CORES)], axis=0)
    return out.astype(np.float32)

